# revision 3
# baseline (speedup 1.0000x reference)
"""Trainium2 Bass kernel for the ExoplanetGNN heterograph message-passing net. V2.

Design (vs baseline):
 - gather tables are PRE-TRANSFORMED by the SAGE linear weights: the planet
   table packs [hp @ 0.5*Wl_sib | hp @ Wl_orb] as [N, 128] bf16 rows (256B);
   the star table holds [hs @ 0.5*Wl_hst] as [N, 64]. The one-hot segment
   matmul then directly yields the applied aggregate, so there is no separate
   stacked-copy + apply matmul.
 - per 512-node dst chunk, ONE PSUM [64, 512] accumulates: hst tiles + sib
   tiles (planets) or orb tiles (stars) + the self-term matmul
   lhsT=Wr_bar rhs=xT_chunk. ScalarE fuses bias+ReLU.
 - node-major table shards are produced by PE matmuls (lhsT=ob[:,128sub],
   rhs=Wcat) -> [128, 64/128] PSUM -> copy -> direct DMA. No dma transposes.
 - one-hot sel built on DVE in fp16 (iota/dr/w fp16) at 2x rate.
 - dst-sharded edges; tables AllGathered at layer boundaries.
"""

import numpy as np
import ml_dtypes

import concourse.bass as bass
import concourse.bacc as bacc
import concourse.mybir as mybir
import concourse.tile as tile
from concourse.bass import IndirectOffsetOnAxis
from concourse.bass_utils import run_bass_kernel_spmd

BF16 = ml_dtypes.bfloat16
F16 = np.float16
BF = mybir.dt.bfloat16
FP16 = mybir.dt.float16
F32 = mybir.dt.float32
I32 = mybir.dt.int32
AF = mybir.ActivationFunctionType
ALU = mybir.AluOpType

C = 8
N_SWDGE_Q = 4
BLK = 128
CHUNK_BLKS = 4     # 512-wide chunks
W = BLK * CHUNK_BLKS
SPAN_COLS = 512


def _patch_indirect_queue():
    import inspect, textwrap
    src_ = textwrap.dedent(inspect.getsource(bass.BassGpSimd.indirect_dma_start))
    src_ = src_.replace("def indirect_dma_start(", "def indirect_dma_start_q(")
    src_ = src_.replace("compute_op: mybir.AluOpType = mybir.AluOpType.bypass,",
                        "compute_op: mybir.AluOpType = mybir.AluOpType.bypass, queue: str = \"qPoolDynamic\",")
    src_ = src_.replace('queue="qPoolDynamic"', "queue=queue")
    ns = vars(bass).copy()
    exec(compile(src_, "<indirect_q>", "exec"), ns)
    bass.BassGpSimd.indirect_dma_start_q = ns["indirect_dma_start_q"]


_patch_indirect_queue()


class Cfg:
    def __init__(self, np_=500000, ns_=200000, fp=32, fs=16, h=64, l=3):
        self.NP, self.NS, self.FP, self.FS, self.H, self.L = np_, ns_, fp, fs, h, l
        assert np_ % C == 0 and ns_ % C == 0
        self.SP, self.SS = np_ // C, ns_ // C
        self.PB = -(-self.SP // BLK)
        self.SB = -(-self.SS // BLK)
        self.NPP, self.NSP = self.PB * BLK, self.SB * BLK
        self.NPT, self.NST = C * self.NPP, C * self.NSP


def _prep_rel(src, dst, src_shard, src_pad, dst_shard, dst_blocks, seg=W):
    """Per-core tile grids for one relation (dst-sharded, seg-wide blocks).

    Returns (srcT [C,128,T] i32, dr [C,128,T] f16, w [C,128,T] f16, tpb)."""
    src = np.asarray(src, np.int64)
    dst = np.asarray(dst, np.int64)
    core = dst // dst_shard
    loc = dst - core * dst_shard
    blk = loc // seg
    rel = loc - blk * seg
    PB = -(-(dst_blocks * BLK) // seg)
    key = core * PB + blk
    cnt = np.bincount(key, minlength=C * PB).reshape(C, PB)
    tpb = np.maximum(1, -(-cnt.max(axis=0) // BLK))
    tile_base = np.concatenate([[0], np.cumsum(tpb)]).astype(np.int64)
    T = int(tile_base[-1])

    order = np.argsort(key, kind="stable")
    key_s = key[order]
    firsts = np.searchsorted(key_s, np.arange(C * PB))
    pos = np.arange(len(key_s)) - firsts[key_s]
    blk_s = blk[order]
    t_idx = tile_base[blk_s] + pos // BLK
    p_idx = pos % BLK
    c_idx = core[order]

    deg = np.bincount(core * dst_shard + loc, minlength=C * dst_shard).astype(np.float64)
    w_e = (1.0 / np.maximum(deg, 1.0))[core * dst_shard + loc]

    s_core = src // src_shard
    s_pad = s_core * src_pad + (src - s_core * src_shard)

    srcT = np.zeros((C, BLK, T), np.int32)
    dr = np.full((C, BLK, T), -1.0, np.float32)
    w = np.zeros((C, BLK, T), np.float32)
    srcT[c_idx, p_idx, t_idx] = s_pad[order]
    dr[c_idx, p_idx, t_idx] = rel[order]
    w[c_idx, p_idx, t_idx] = w_e[order]
    return srcT, dr, w, tpb.tolist()


def _spans(tpb):
    """Spans of <= SPAN_COLS tile-columns aligned to chunk (block) boundaries."""
    tile_base = [0]
    for t in tpb:
        tile_base.append(tile_base[-1] + t)
    spans, soc = [], []
    cur0 = 0
    for b in range(len(tpb)):
        t0, t1 = tile_base[b], tile_base[b + 1]
        if t1 - cur0 > SPAN_COLS and t0 > cur0:
            spans.append((cur0, t0))
            cur0 = t0
        soc.append(len(spans))
    spans.append((cur0, tile_base[-1]))
    return tile_base, spans, soc


def build(cfg, grids, b2val):
    H, FP, FS, L = cfg.H, cfg.FP, cfg.FS, cfg.L
    nc = bacc.Bacc(None, target_bir_lowering=False, num_devices=C, num_swdge_queues=4)

    def param(name, shape, dt):
        return nc.declare_dram_parameter(name, shape, dt, isOutput=False)

    xpt = param("xpt", [FP, cfg.NPP], BF)
    xst = param("xst", [FS, cfg.NSP], BF)
    eparams = {}
    for r in ("orb", "hst", "sib"):
        T = grids[r][4]
        eparams[r] = (
            param(f"{r}_src", [BLK, T], I32),
            param(f"{r}_dr", [BLK, T], F32),
            param(f"{r}_w", [BLK, T], F32),
        )
    iota_p = param("iota", [128, W], F32)
    wp_p = param("wp", [FP, H], BF)
    bp_p = param("bp", [H, 1], F32)
    ws_p = param("ws", [FS, H], BF)
    bs_p = param("bs", [H, 1], F32)
    # per-layer: wcat_p[l] = [0.5*Wl[l,2] | Wl[l,0]] (planet table halves), [64, 128]
    wcat_p_p = param("wcat_p", [L, H, 2 * H], BF)
    wsib2_p = param("wsib2", [H, H], BF)          # 0.5*Wl[2,2] for compact last table
    whst_p = param("whst", [L, H, H], BF)         # 0.5*Wl[l,1]
    wrp_p = param("wrp", [L, H, H], BF)           # 0.5*(Wr[l,1]+Wr[l,2])
    wrs_p = param("wrs", [L, H, H], BF)           # Wr[l,0]
    bias_s_p = param("bias_s", [L, H, 1], F32)    # bl[l,0]
    bias_p_p = param("bias_p", [L, H, 1], F32)    # 0.5*(bl[l,1]+bl[l,2])
    w1_p = param("w1", [H, H // 2], BF)
    b1_p = param("b1", [H // 2, 1], F32)
    w2_p = param("w2", [H // 2, 1], BF)
    out_p = nc.declare_dram_parameter("out", [1, cfg.NPP], F32, isOutput=True)

    npchunks = -(-cfg.PB // CHUNK_BLKS)
    nschunks = -(-cfg.SB // CHUNK_BLKS)
    pchunks = [(c * CHUNK_BLKS, min(CHUNK_BLKS, cfg.PB - c * CHUNK_BLKS)) for c in range(npchunks)]
    schunks = [(c * CHUNK_BLKS, min(CHUNK_BLKS, cfg.SB - c * CHUNK_BLKS)) for c in range(nschunks)]

    with tile.TileContext(nc) as tc:
        with (
            tc.tile_pool(name="const", bufs=1) as cp,
            tc.tile_pool(name="dram", bufs=1, space="DRAM") as dp,
            tc.tile_pool(name="idx", bufs=4) as ip,
            tc.tile_pool(name="msg", bufs=3) as mp,
            tc.tile_pool(name="sel", bufs=24) as selp,
            tc.tile_pool(name="work", bufs=4) as wkp,
            tc.tile_pool(name="psum", bufs=1, space="PSUM") as pp,
        ):
            # ---- persistent DRAM state ----
            # packed planet tables (one per layer it feeds): [NPT, 128]
            tp_tab = [
                dp.tile([cfg.NPT, 2 * H], BF, addr_space="Shared", tag=f"tp{i}", name=f"tp{i}")
                for i in range(2)  # layers 0,1 use packed; layer 2 compact
            ]
            tp2_tab = dp.tile([cfg.NPT, H], BF, addr_space="Shared", tag="tp2", name="tp2")
            ts_tab = [
                dp.tile([cfg.NST, H], BF, addr_space="Shared", tag=f"ts{i}", name=f"ts{i}")
                for i in range(L)
            ]
            tp_shard = dp.tile([cfg.NPP, 2 * H], BF, tag="tp_shard")
            tp2_shard = dp.tile([cfg.NPP, H], BF, tag="tp2_shard")
            ts_shard = dp.tile([cfg.NSP, H], BF, tag="ts_shard")
            xpT = [dp.tile([H, cfg.NPP], BF, tag=f"xpT{i}", name=f"xpT{i}") for i in range(2)]
            xsT = [dp.tile([H, cfg.NSP], BF, tag=f"xsT{i}", name=f"xsT{i}") for i in range(2)]

            # ---- consts ----
            def ld(shape, dt, src_ap, tag):
                t = cp.tile(shape, dt, tag=tag)
                nc.sync.dma_start(out=t[:], in_=src_ap)
                return t

            iota_t = ld([128, W], F32, iota_p[:, :], "iota")
            wp_t = ld([FP, H], BF, wp_p[:, :], "wp")
            ws_t = ld([FS, H], BF, ws_p[:, :], "ws")
            bp_t = ld([H, 1], F32, bp_p[:, :], "bp")
            bs_t = ld([H, 1], F32, bs_p[:, :], "bs")
            w1_t = ld([H, H // 2], BF, w1_p[:, :], "w1")
            b1_t = ld([H // 2, 1], F32, b1_p[:, :], "b1")
            w2_t = ld([H // 2, 1], BF, w2_p[:, :], "w2")
            wsib2_t = ld([H, H], BF, wsib2_p[:, :], "wsib2")
            wcat_t = [ld([H, 2 * H], BF, wcat_p_p[l, :, :], f"wcat{l}") for l in range(L)]
            whst_t = [ld([H, H], BF, whst_p[l, :, :], f"whst{l}") for l in range(L)]
            wrp_t = [ld([H, H], BF, wrp_p[l, :, :], f"wrp{l}") for l in range(L)]
            wrs_t = [ld([H, H], BF, wrs_p[l, :, :], f"wrs{l}") for l in range(L)]
            bias_s_t = [ld([H, 1], F32, bias_s_p[l, :, :], f"bss{l}") for l in range(L)]
            bias_p_t = [ld([H, 1], F32, bias_p_p[l, :, :], f"bsp{l}") for l in range(L)]

            def allgather(shard, tab):
                nc.gpsimd.collective_compute(
                    "AllGather", ALU.bypass,
                    replica_groups=[list(range(C))],
                    ins=[shard[:, :]], outs=[tab[:, :]],
                )

            class SpanState:
                def __init__(self, rel):
                    self.rel = rel
                    self.cur = -1
                    self.tiles = None

                def ensure(self, si, spans):
                    if self.cur == si:
                        return
                    self.cur = si
                    t0, t1 = spans[si]
                    n = t1 - t0
                    sp, dp_, wp_ = eparams[self.rel]
                    st = ip.tile([BLK, n], I32, tag=f"{self.rel}_src", name="st")
                    nc.sync.dma_start(out=st[:], in_=sp[:, t0:t1])
                    dt_ = ip.tile([BLK, n], F32, tag=f"{self.rel}_dr", name="dt")
                    nc.sync.dma_start(out=dt_[:], in_=dp_[:, t0:t1])
                    wt = ip.tile([BLK, n], F32, tag=f"{self.rel}_w", name="wt")
                    nc.sync.dma_start(out=wt[:], in_=wp_[:, t0:t1])
                    self.tiles = (st, dt_, wt, t0)

            def agg_chunk(rel, state, table, rowlen, lh0, ci, cw, agg_psum, start):
                """Gather + one-hot + segment matmuls for chunk ci of relation rel.
                table rows are rowlen bf16 elems; lhsT slice = cols [lh0, lh0+H)."""
                tile_base, tpb, spans, soc, T = grids[rel]
                state.ensure(soc[ci], spans)
                st, dt_, wt, t0 = state.tiles
                c_t0, c_t1 = tile_base[ci], tile_base[ci + 1]
                gk = c_t1 - c_t0
                msg = mp.tile([128, gk * rowlen], BF, tag=f"{rel}_msg", bufs=6,
                              name="msg")
                for j in range(gk):
                    t = c_t0 + j
                    jj = t - t0
                    qn = t % N_SWDGE_Q
                    nc.gpsimd.indirect_dma_start_q(
                        out=msg[:, j * rowlen: (j + 1) * rowlen],
                        out_offset=None,
                        in_=table[:, :],
                        in_offset=IndirectOffsetOnAxis(ap=st[:, jj: jj + 1], axis=0),
                        queue=f"qPoolDynamic{qn or ''}",
                    )
                for j in range(gk):
                    t = c_t0 + j
                    jj = t - t0
                    sel = selp.tile([128, cw], BF, tag="sel", name="sel")
                    nc.vector.tensor_scalar(
                        out=sel[:],
                        in0=iota_t[:, :cw],
                        scalar1=dt_[:, jj: jj + 1],
                        scalar2=wt[:, jj: jj + 1],
                        op0=ALU.is_equal,
                        op1=ALU.mult,
                    )
                    nc.tensor.matmul(
                        out=agg_psum[:, :cw],
                        lhsT=msg[:, j * rowlen + lh0: j * rowlen + lh0 + H],
                        rhs=sel[:],
                        start=(start and j == 0),
                        stop=False,
                    )

            def tprod(ob, c0, cw, wtile, wcols, shard):
                """Node-major table production: per 128-sub matmul + copy + DMA."""
                nsub = cw // BLK
                for i in range(nsub):
                    pt = pp.tile([BLK, 2 * H], F32, tag="tp", bufs=2, name="pt")
                    nc.tensor.matmul(
                        out=pt[:, :wcols],
                        lhsT=ob[:, i * BLK: (i + 1) * BLK],
                        rhs=wtile[:],
                        start=True, stop=True,
                    )
                    nm = wkp.tile([BLK, 2 * H], BF, tag="nm", bufs=6, name="nm")
                    nc.scalar.activation(out=nm[:, :wcols], in_=pt[:, :wcols], func=AF.Copy)
                    r0 = c0 + i * BLK
                    nc.sync.dma_start(out=shard[r0: r0 + BLK, :], in_=nm[:, :wcols])

            # =================== input projection ===================
            for (b0, nb) in pchunks:
                cw, c0 = nb * BLK, b0 * BLK
                xp = wkp.tile([FP, W], BF, tag="xp", name="xp")
                nc.sync.dma_start(out=xp[:, :cw], in_=xpt[:, c0: c0 + cw])
                po = pp.tile([H, W], F32, tag="agg", bufs=2, name="po")
                nc.tensor.matmul(out=po[:, :cw], lhsT=wp_t[:], rhs=xp[:, :cw], start=True, stop=True)
                ob = wkp.tile([H, W], BF, tag="ob", name="ob")
                nc.scalar.activation(out=ob[:, :cw], in_=po[:, :cw], func=AF.Relu, bias=bp_t[:], scale=1.0)
                nc.sync.dma_start(out=xpT[0][:, c0: c0 + cw], in_=ob[:, :cw])
                tprod(ob, c0, cw, wcat_t[0], 2 * H, tp_shard)
            # planet table allgather overlaps the star projection
            allgather(tp_shard, tp_tab[0])
            for (b0, nb) in schunks:
                cw, c0 = nb * BLK, b0 * BLK
                xs = wkp.tile([FS, W], BF, tag="xs", name="xs")
                nc.sync.dma_start(out=xs[:, :cw], in_=xst[:, c0: c0 + cw])
                po = pp.tile([H, W], F32, tag="agg", bufs=2, name="po")
                nc.tensor.matmul(out=po[:, :cw], lhsT=ws_t[:], rhs=xs[:, :cw], start=True, stop=True)
                ob = wkp.tile([H, W], BF, tag="ob", name="ob")
                nc.scalar.activation(out=ob[:, :cw], in_=po[:, :cw], func=AF.Relu, bias=bs_t[:], scale=1.0)
                nc.sync.dma_start(out=xsT[0][:, c0: c0 + cw], in_=ob[:, :cw])
                tprod(ob, c0, cw, whst_t[0], H, ts_shard)
            allgather(ts_shard, ts_tab[0])

            # =================== SAGE layers ===================
            for l in range(L):
                rp, wpar = l % 2, (l + 1) % 2
                # ---- stars (skip at last layer) ----
                if l < L - 1:
                    st_orb = SpanState("orb")
                    for ci, (b0, nb) in enumerate(schunks):
                        cw, c0 = nb * BLK, b0 * BLK
                        agg = pp.tile([H, W], F32, tag="agg", bufs=2, name="agg")
                        agg_chunk("orb", st_orb, tp_tab[l], 2 * H, H, ci, cw, agg, True)
                        xt = wkp.tile([H, W], BF, tag="xt", name="xt")
                        nc.sync.dma_start(out=xt[:, :cw], in_=xsT[rp][:, c0: c0 + cw])
                        nc.tensor.matmul(out=agg[:, :cw], lhsT=wrs_t[l][:], rhs=xt[:, :cw],
                                         start=False, stop=True)
                        ob = wkp.tile([H, W], BF, tag="ob", name="ob")
                        nc.scalar.activation(out=ob[:, :cw], in_=agg[:, :cw], func=AF.Relu,
                                             bias=bias_s_t[l][:], scale=1.0)
                        nc.sync.dma_start(out=xsT[wpar][:, c0: c0 + cw], in_=ob[:, :cw])
                        tprod(ob, c0, cw, whst_t[l + 1], H, ts_shard)
                    allgather(ts_shard, ts_tab[l + 1])
                # ---- planets ----
                st_hst = SpanState("hst")
                st_sib = SpanState("sib")
                ptab, prow, plh0 = (
                    (tp_tab[l], 2 * H, 0) if l < L - 1 else (tp2_tab, H, 0)
                )
                for ci, (b0, nb) in enumerate(pchunks):
                    cw, c0 = nb * BLK, b0 * BLK
                    agg = pp.tile([H, W], F32, tag="agg", bufs=2, name="agg")
                    agg_chunk("hst", st_hst, ts_tab[l], H, 0, ci, cw, agg, True)
                    agg_chunk("sib", st_sib, ptab, prow, plh0, ci, cw, agg, False)
                    xt = wkp.tile([H, W], BF, tag="xt", name="xt")
                    nc.sync.dma_start(out=xt[:, :cw], in_=xpT[rp][:, c0: c0 + cw])
                    nc.tensor.matmul(out=agg[:, :cw], lhsT=wrp_t[l][:], rhs=xt[:, :cw],
                                     start=False, stop=True)
                    ob = wkp.tile([H, W], BF, tag="ob", name="ob")
                    nc.scalar.activation(out=ob[:, :cw], in_=agg[:, :cw], func=AF.Relu,
                                         bias=bias_p_t[l][:], scale=1.0)
                    if l < L - 1:
                        nc.sync.dma_start(out=xpT[wpar][:, c0: c0 + cw], in_=ob[:, :cw])
                        if l == 0:
                            tprod(ob, c0, cw, wcat_t[1], 2 * H, tp_shard)
                        else:
                            tprod(ob, c0, cw, wsib2_t, H, tp2_shard)
                    else:
                        pr = pp.tile([H // 2, W], F32, tag="r1", bufs=1, name="pr")
                        nc.tensor.matmul(out=pr[:, :cw], lhsT=w1_t[:], rhs=ob[:, :cw],
                                         start=True, stop=True)
                        r1 = wkp.tile([H // 2, W], BF, tag="r1sb", name="r1")
                        nc.scalar.activation(out=r1[:, :cw], in_=pr[:, :cw], func=AF.Relu,
                                             bias=b1_t[:], scale=1.0)
                        py = pp.tile([1, W], F32, tag="y", bufs=1, name="py")
                        nc.tensor.matmul(out=py[:, :cw], lhsT=w2_t[:], rhs=r1[:, :cw],
                                         start=True, stop=True)
                        ysb = wkp.tile([1, W], F32, tag="ysb", name="ysb")
                        nc.vector.tensor_scalar_add(out=ysb[:, :cw], in0=py[:, :cw],
                                                    scalar1=float(b2val))
                        nc.sync.dma_start(out=out_p[0:1, c0: c0 + cw], in_=ysb[:, :cw])
                if l == 0:
                    allgather(tp_shard, tp_tab[1])
                elif l == 1:
                    allgather(tp2_shard, tp2_tab)

    nc.finalize()
    return nc


def _prep_all(inputs, cfg):
    f32 = np.float32
    xp = np.asarray(inputs["x_planet"], f32)
    xs = np.asarray(inputs["x_star"], f32)
    Wp = np.asarray(inputs["Wp"], f32)
    bp = np.asarray(inputs["bp"], f32)
    Ws = np.asarray(inputs["Ws"], f32)
    bs = np.asarray(inputs["bs"], f32)
    Wl = np.asarray(inputs["Wl"], f32)
    bl = np.asarray(inputs["bl"], f32)
    Wr = np.asarray(inputs["Wr"], f32)
    W1 = np.asarray(inputs["W1"], f32)
    b1 = np.asarray(inputs["b1"], f32)
    W2 = np.asarray(inputs["W2"], f32)
    b2 = np.asarray(inputs["b2"], f32)

    orb = _prep_rel(inputs["orbits_src"], inputs["orbits_dst"],
                    cfg.SP, cfg.NPP, cfg.SS, cfg.SB)
    hst = _prep_rel(inputs["hosts_src"], inputs["hosts_dst"],
                    cfg.SS, cfg.NSP, cfg.SP, cfg.PB)
    sib = _prep_rel(inputs["sib_src"], inputs["sib_dst"],
                    cfg.SP, cfg.NPP, cfg.SP, cfg.PB)

    grids = {}
    for name, r in (("orb", orb), ("hst", hst), ("sib", sib)):
        tpb = r[3]
        tile_base, spans, soc = _spans(tpb)
        grids[name] = (tile_base, tpb, spans, soc, tile_base[-1])

    L, H = cfg.L, cfg.H
    # wcat_p[l] = [0.5*Wl[l,2] | Wl[l,0]]  (sib half | orb half)
    wcat = np.stack([np.concatenate([0.5 * Wl[l, 2], Wl[l, 0]], axis=1) for l in range(L)])
    whst = np.stack([0.5 * Wl[l, 1] for l in range(L)])
    wrp = np.stack([0.5 * (Wr[l, 1] + Wr[l, 2]) for l in range(L)])
    wrs = np.stack([Wr[l, 0] for l in range(L)])
    bias_s = np.stack([bl[l, 0][:, None] for l in range(L)])
    bias_p = np.stack([0.5 * (bl[l, 1] + bl[l, 2])[:, None] for l in range(L)])
    iota = np.tile(np.arange(W, dtype=np.float32), (128, 1))

    common = {
        "iota": iota,
        "wp": Wp.astype(BF16), "bp": bp[:, None],
        "ws": Ws.astype(BF16), "bs": bs[:, None],
        "wcat_p": wcat.astype(BF16),
        "wsib2": (0.5 * Wl[2, 2]).astype(BF16),
        "whst": whst.astype(BF16),
        "wrp": wrp.astype(BF16), "wrs": wrs.astype(BF16),
        "bias_s": bias_s, "bias_p": bias_p,
        "w1": W1.astype(BF16), "b1": b1[:, None], "w2": W2.astype(BF16),
    }
    in_maps = []
    for c in range(C):
        xpt_c = np.zeros((cfg.FP, cfg.NPP), BF16)
        xpt_c[:, : cfg.SP] = xp[c * cfg.SP: (c + 1) * cfg.SP].T.astype(BF16)
        xst_c = np.zeros((cfg.FS, cfg.NSP), BF16)
        xst_c[:, : cfg.SS] = xs[c * cfg.SS: (c + 1) * cfg.SS].T.astype(BF16)
        m = dict(common)
        m["xpt"] = xpt_c
        m["xst"] = xst_c
        for name, r in (("orb", orb), ("hst", hst), ("sib", sib)):
            m[f"{name}_src"] = r[0][c]
            m[f"{name}_dr"] = r[1][c]
            m[f"{name}_w"] = r[2][c]
        in_maps.append(m)
    return in_maps, grids, float(b2[0])


LAST_RESULT = None


def kernel(_cfg=None, _trace=False, **inputs):
    global LAST_RESULT
    cfg = _cfg or Cfg()
    in_maps, grids_, b2val = _prep_all(inputs, cfg)
    global grids
    grids = grids_
    nc = build(cfg, grids_, b2val)
    res = run_bass_kernel_spmd(nc, in_maps, list(range(C)), trace=_trace)
    LAST_RESULT = res
    out = np.concatenate(
        [res.results[c]["out"][0, : cfg.SP] for c in range(C)]
    ).astype(np.float32)
    return out
